# revision 21
# baseline (speedup 1.0000x reference)
"""GATv2 (2-layer, 8-head) message-passing kernel for 8 Trainium2 NeuronCores.

v2: nodes are bin-packed into 40 dst-blocks of 32 nodes per core so every
block has <=1024 incoming edges -> uniform 8 tiles/block, ONE 1024-idx
dma_gather per block. Gather payload is plain xl (256 bf16 cols, 512B rows);
the LeakyReLU is computed directly on the Scalar engine (Lrelu, alpha=0.2),
so no linear-term augmentation is needed (the dst-side linear part need not
cancel: softmax is shift-invariant per dst, and we keep e exact anyway).
xr and the replicated h0 stay resident in SBUF; the xsrc add into PSUM is
done with 2 grouped identity matmuls (512 cols) instead of 8.

Self-contained: takes full (unsharded) inputs, returns the full output.
"""

import os
import numpy as np
from contextlib import ExitStack

import ml_dtypes
import concourse.bass as bass
import concourse.tile as tile
from concourse import bacc, mybir
from concourse.bass_utils import run_bass_kernel_spmd

N = 10000
E = 320000
F_IN = 7
D = 256
H = 8
C = 32
L = 2
G = 16
SLOPE = 0.2

NCORES = 8
NPC_REAL = 1250
NPC = 1280
NP = NCORES * NPC
BLK = 32
NBLK = NPC // BLK        # 40
PT = 128

F32 = mybir.dt.float32
BF16 = mybir.dt.bfloat16
I16 = mybir.dt.int16
FP8 = mybir.dt.float8e4
NBF = ml_dtypes.bfloat16
NF8 = ml_dtypes.float8_e4m3
WSC = 16.0   # fp8 weight scale (cancelled via att/WSC and denom scale)


def _prep_edges(edge_index: np.ndarray):
    """Bin-pack dst nodes into 40 blocks/core, relabel local node ids so
    block b = new-local nodes [32b, 32b+32). Returns (t_blk, pm, cores)."""
    src_g, dst_g = edge_index[0], edge_index[1]
    deg_all = np.zeros(N, dtype=np.int64)
    np.add.at(deg_all, dst_g, 1)

    pm = np.zeros(N, dtype=np.int64)      # original global -> padded new id
    for c in range(NCORES):
        deg = deg_all[c * NPC_REAL:(c + 1) * NPC_REAL]
        order = np.argsort(-deg, kind="stable")
        sums = np.zeros(NBLK)
        fill = np.zeros(NBLK, dtype=np.int64)
        newloc = np.zeros(NPC_REAL, dtype=np.int64)
        for n in order:
            cand = np.flatnonzero(fill < BLK)
            b = cand[np.argmin(sums[cand])]
            newloc[n] = b * BLK + fill[b]
            sums[b] += deg[n]
            fill[b] += 1
        pm[c * NPC_REAL:(c + 1) * NPC_REAL] = c * NPC + newloc

    owner = dst_g // NPC_REAL
    t_blk = 8
    per_core = []
    for c in range(NCORES):
        sel = owner == c
        es = pm[src_g[sel]]
        ed = pm[dst_g[sel]] - c * NPC
        cnt = np.bincount(ed // BLK, minlength=NBLK)
        t_blk = max(t_blk, int(np.ceil(cnt.max() / PT)))
        per_core.append((es, ed))

    spb = t_blk * PT                     # slots per block
    ne_slots = NBLK * spb
    cores = []
    for c in range(NCORES):
        es, ed = per_core[c]
        blk = ed // BLK
        order = np.argsort(blk, kind="stable")
        es, ed, blk = es[order], ed[order], blk[order]
        cnt = np.bincount(blk, minlength=NBLK)
        starts = np.zeros(NBLK, dtype=np.int64)
        starts[1:] = np.cumsum(cnt)[:-1]
        within = np.arange(len(es)) - starts[blk]
        slot = blk * spb + within

        src_slots = np.zeros(ne_slots, dtype=np.int64)
        dl32 = np.full(ne_slots, -1, dtype=np.int64)
        src_slots[slot] = es
        dl32[slot] = ed % BLK

        w16 = np.concatenate(
            [src_slots[b * spb:(b + 1) * spb].reshape(spb // 16, 16).T
             for b in range(NBLK)], axis=1).astype(np.int16)
        srcw = np.tile(w16, (8, 1))

        nt = NBLK * t_blk
        ind = np.zeros((ne_slots, BLK), dtype=np.float32)
        valid = dl32 >= 0
        ind[np.nonzero(valid)[0], dl32[valid]] = 1.0
        ind = ind.reshape(nt, PT, BLK)
        ind_pm = np.ascontiguousarray(ind.transpose(1, 0, 2)).astype(NBF)
        indt_pm = np.ascontiguousarray(
            ind.transpose(0, 2, 1).transpose(1, 0, 2)).astype(NBF)
        cores.append({"srcw": srcw, "ind_pm": ind_pm, "indt_pm": indt_pm})
    return t_blk, pm, cores


def _build(t_blk: int):
    STAGE = int(os.environ.get("GAT_STAGE", "99"))
    LRELU = os.environ.get("GAT_LRELU", "1") == "1"
    nt = NBLK * t_blk
    ne_slots = nt * PT
    QW = 4

    nc = bacc.Bacc("TRN2", target_bir_lowering=False, debug=False,
                   num_devices=NCORES)

    xT = nc.dram_tensor("xT", [F_IN, NP], F32, kind="ExternalInput").ap()
    xTown = nc.dram_tensor("xTown", [F_IN, NPC], F32, kind="ExternalInput").ap()
    wp = nc.dram_tensor("wp", [F_IN, D], F32, kind="ExternalInput").ap()
    wl = nc.dram_tensor("wl", [128, L, 2, D], FP8, kind="ExternalInput").ap()
    wr = nc.dram_tensor("wr", [128, L, 2, D], FP8, kind="ExternalInput").ap()
    attrep = nc.dram_tensor("attrep", [128, L, QW, D], BF16, kind="ExternalInput").ap()
    bcols = nc.dram_tensor("bcols", [128, 2 + 2 * L + 2], F32, kind="ExternalInput").ap()
    i128 = nc.dram_tensor("i128", [128, 128], BF16, kind="ExternalInput").ap()
    i32 = nc.dram_tensor("i32", [32, 32], F32, kind="ExternalInput").ap()
    epsc = nc.dram_tensor("epsc", [32, 1], F32, kind="ExternalInput").ap()
    srcw_d = nc.dram_tensor("srcw", [128, ne_slots // 16], I16, kind="ExternalInput").ap()
    ind_d = nc.dram_tensor("ind", [128, nt, BLK], BF16, kind="ExternalInput").ap()
    indt_d = nc.dram_tensor("indt", [32, nt, PT], BF16, kind="ExternalInput").ap()

    xl_d = nc.dram_tensor("xl_d", [NP, D], BF16).ap()
    xr_d = nc.dram_tensor("xr_d", [NPC, D], BF16).ap()
    NCHK = 5
    CHN = NPC // NCHK
    h_upd = nc.dram_tensor("h_upd", [2, 128, NPC], FP8).ap()
    h_all = nc.dram_tensor("h_all", [NCORES, 2, 128, NPC], FP8,
                           addr_space="Shared").ap()

    y_out = nc.dram_tensor("y_out", [1, NPC], F32, kind="ExternalOutput").ap()

    with tile.TileContext(nc) as tc, ExitStack() as ctx:
        cpool = ctx.enter_context(tc.tile_pool(name="consts", bufs=1))
        stream = ctx.enter_context(tc.tile_pool(name="stream", bufs=4))
        drain = ctx.enter_context(tc.tile_pool(name="drain", bufs=4))
        gath = ctx.enter_context(tc.tile_pool(name="gath", bufs=3))
        indp = ctx.enter_context(tc.tile_pool(name="indp", bufs=3))
        indtp = ctx.enter_context(tc.tile_pool(name="indtp", bufs=3))
        xrp = ctx.enter_context(tc.tile_pool(name="xrp", bufs=3))
        upool = ctx.enter_context(tc.tile_pool(name="upool", bufs=3))
        appool = ctx.enter_context(tc.tile_pool(name="appool", bufs=3))
        epool = ctx.enter_context(tc.tile_pool(name="epool", bufs=3))
        astp = ctx.enter_context(tc.tile_pool(name="astp", bufs=3))
        smallp = ctx.enter_context(tc.tile_pool(name="smallp", bufs=3))
        outbp = ctx.enter_context(tc.tile_pool(name="outbp", bufs=3))

        pfeat = ctx.enter_context(tc.tile_pool(name="pfeat", bufs=2, space="PSUM"))
        pblk = ctx.enter_context(tc.tile_pool(name="pblk", bufs=3, space="PSUM"))
        ptrp = ctx.enter_context(tc.tile_pool(name="ptrp", bufs=1, space="PSUM"))

        def pmm_tile():
            t = pfeat.tile([128, 2, 512], F32, tag="pf")
            return t[:, 0, :]

        wp_sb = cpool.tile([F_IN, D], F32)
        nc.sync.dma_start(wp_sb[:], wp)
        wl_sb = cpool.tile([128, L, 2, D], FP8)
        nc.sync.dma_start(wl_sb[:], wl)
        wr_sb = cpool.tile([128, L, 2, D], FP8)
        nc.sync.dma_start(wr_sb[:], wr)
        att_sb = cpool.tile([128, L, QW, D], BF16)
        nc.sync.dma_start(att_sb[:], attrep)
        bc_sb = cpool.tile([128, 2 + 2 * L + 2], F32)
        nc.sync.dma_start(bc_sb[:], bcols)
        i128_sb = cpool.tile([128, 128], BF16)
        nc.sync.dma_start(i128_sb[:], i128)
        i32_sb = cpool.tile([32, 32], F32)
        nc.sync.dma_start(i32_sb[:], i32)
        eps_sb = cpool.tile([32, 1], F32)
        nc.sync.dma_start(eps_sb[:], epsc)
        srcw_sb = cpool.tile([128, ne_slots // 16], I16)
        nc.sync.dma_start(srcw_sb[:], srcw_d)

        h_own = cpool.tile([128, 2, NPC], F32)    # own h, fp32, resident
        h_bf = cpool.tile([128, 2, NPC], FP8)    # own h, bf16 (xr stationary)
        h0_sb = cpool.tile([128, 2, NP], FP8)    # replicated h0 (layer-0 xl)

        def drain_ps(dst_ap, ps_ap, parity, bias=None):
            if bias is not None:
                if parity % 2 == 0:
                    nc.scalar.activation(dst_ap, ps_ap,
                                         mybir.ActivationFunctionType.Identity,
                                         bias=bias)
                else:
                    nc.vector.tensor_scalar_add(dst_ap, ps_ap, bias)
            elif parity % 2 == 0:
                nc.scalar.activation(dst_ap, ps_ap,
                                     mybir.ActivationFunctionType.Copy)
            else:
                nc.vector.tensor_copy(dst_ap, ps_ap)

        # ---- P0: h0 = x @ Wp + bp (replicated) -> h0_sb (bf16, SBUF) ----
        for b in range(NCORES):
            for k in range(4):
                xs = stream.tile([F_IN, 320], F32, tag="xs")
                nc.sync.dma_start(xs[:], xT[:, b * NPC + 320 * k:b * NPC + 320 * (k + 1)])
                for ch in range(2):
                    ps = pmm_tile()
                    nc.tensor.matmul(ps[:, 0:320], wp_sb[:, 128 * ch:128 * (ch + 1)],
                                     xs[:], start=True, stop=True)
                    drain_ps(h0_sb[:, ch, b * NPC + 320 * k:b * NPC + 320 * (k + 1)],
                             ps[:, 0:320], b * 8 + k * 2 + ch,
                             bias=bc_sb[:, ch:ch + 1])
        # P0b: own slice -> h_own (f32) + h_bf (bf16)
        for k in range(4):
            xs = stream.tile([F_IN, 320], F32, tag="xs")
            nc.sync.dma_start(xs[:], xTown[:, 320 * k:320 * (k + 1)])
            for ch in range(2):
                ps = pmm_tile()
                nc.tensor.matmul(ps[:, 0:320], wp_sb[:, 128 * ch:128 * (ch + 1)],
                                 xs[:], start=True, stop=True)
                drain_ps(h_own[:, ch, 320 * k:320 * (k + 1)], ps[:, 0:320], 0,
                         bias=bc_sb[:, ch:ch + 1])
                nc.vector.tensor_scalar_add(h_bf[:, ch, 320 * k:320 * (k + 1)],
                                            ps[:, 0:320], bc_sb[:, ch:ch + 1])

        # ---- layers ----
        for l in range(L if STAGE >= 2 else 0):
            # xl = h @ Wl  -> xl_d (node-major bf16)
            # jp <-> (bb, k): rows 256*jp = core bb = jp//5, chunk k = jp%5
            jp_order = (range(40) if l == 0 else
                        [5 * bb + k for k in range(5) for bb in range(8)])
            for jp in jp_order:
                ps2 = pfeat.tile([128, 2, 512], F32, tag="pf")
                if l == 0:
                    for o in range(2):
                        j = 2 * jp + o
                        nc.tensor.matmul(ps2[:, o, 0:D],
                                         h0_sb[:, :, 128 * j:128 * (j + 1)],
                                         wl_sb[:, l, :, :],
                                         start=True, stop=True,
                                         perf_mode=mybir.MatmulPerfMode.DoubleRow)
                else:
                    bb, lc = (2 * jp) // 10, ((2 * jp) % 10) * 128
                    ht = stream.tile([128, 2, 256], FP8, tag="ht")
                    nc.sync.dma_start(
                        ht[:], h_all[bb, :, :, lc:lc + 256].rearrange("c p n -> p c n"))
                    for o in range(2):
                        nc.tensor.matmul(ps2[:, o, 0:D],
                                         ht[:, :, 128 * o:128 * (o + 1)],
                                         wl_sb[:, l, :, :],
                                         start=True, stop=True,
                                         perf_mode=mybir.MatmulPerfMode.DoubleRow)
                xsb = drain.tile([128, 2, D], BF16, tag="xsb")
                drain_ps(xsb[:], ps2[:, :, 0:D], jp)
                row0 = 2 * jp * 128
                nc.sync.dma_start(
                    xl_d[row0:row0 + 256, :].rearrange("(a p) d -> p a d", p=128),
                    xsb[:])

            # xr = h_own @ Wr -> xr_d (node-major bf16, DRAM round-trip)
            for jo in range(NPC // 128 if STAGE >= 3 else 0):
                ps = pmm_tile()
                nc.tensor.matmul(ps[:, 0:D], h_bf[:, :, 128 * jo:128 * (jo + 1)],
                                 wr_sb[:, l, :, :],
                                 start=True, stop=True,
                                 perf_mode=mybir.MatmulPerfMode.DoubleRow)
                xrsb = drain.tile([128, D], BF16, tag="xrsb")
                drain_ps(xrsb[:], ps[:, 0:D], jo)
                nc.sync.dma_start(xr_d[128 * jo:128 * (jo + 1), :], xrsb[:])

            # edge sweep: gather 2 blocks per call (2048 idxs)
            nblk_run = NBLK if STAGE >= 4 else 0

            def finalize_chunk(k):
                """bias + bf16-cast + h_upd store for 256-node chunk k."""
                n0 = CHN * k
                for ch in range(2):
                    nc.vector.tensor_scalar_add(
                        h_own[:, ch, n0:n0 + CHN], h_own[:, ch, n0:n0 + CHN],
                        bc_sb[:, 2 + 2 * l + ch:3 + 2 * l + ch])
                if l == 0 and STAGE >= 5:
                    nc.scalar.activation(h_bf[:, 0, n0:n0 + CHN],
                                         h_own[:, 0, n0:n0 + CHN],
                                         mybir.ActivationFunctionType.Copy)
                    nc.vector.tensor_copy(h_bf[:, 1, n0:n0 + CHN],
                                          h_own[:, 1, n0:n0 + CHN])
                    nc.sync.dma_start(
                        h_upd[:, :, n0:n0 + CHN].rearrange("c p n -> p c n"),
                        h_bf[:, :, n0:n0 + CHN])

            BPC = NBLK // NCHK           # blocks per finalize chunk (8)
            GB = int(os.environ.get("GAT_GB", "2"))   # blocks per gather call
            for bp2 in range(nblk_run // GB):
                xsrc2 = gath.tile([128, GB * t_blk, D], BF16)
                for gs in range(0, GB * t_blk * PT, 2048):
                    gw = min(2048, GB * t_blk * PT - gs)
                    nc.gpsimd.dma_gather(
                        out_ap=xsrc2[:, gs // PT:(gs + gw) // PT, :],
                        in_ap=xl_d,
                        idxs_ap=srcw_sb[:, (bp2 * GB * t_blk * PT + gs) // 16:
                                        (bp2 * GB * t_blk * PT + gs + gw) // 16],
                        num_idxs=gw,
                        num_idxs_reg=gw,
                        elem_size=D,
                    )
                for sub in range(GB):
                    b = GB * bp2 + sub
                    xsrc = xsrc2[:, sub * t_blk:(sub + 1) * t_blk, :]
                    ind_sb = indp.tile([128, t_blk, BLK], BF16)
                    nc.sync.dma_start(ind_sb[:], ind_d[:, b * t_blk:(b + 1) * t_blk, :])
                    indt_sb = indtp.tile([32, t_blk, PT], BF16)
                    nc.sync.dma_start(indt_sb[:], indt_d[:, b * t_blk:(b + 1) * t_blk, :])
                    xrb_t = xrp.tile([32, D], BF16)
                    nc.sync.dma_start(xrb_t[:], xr_d[BLK * b:BLK * (b + 1), :])
                    xrb = xrb_t[:]

                    ast = astp.tile([128, t_blk, 8 + D], BF16)
                    pb = pblk.tile([32, 8 + D], F32)

                    for q0 in range(0, t_blk, QW):
                        w = min(QW, t_blk - q0)
                        pf0 = pfeat.tile([128, 2, 512], F32, tag="pf")
                        pf = pf0[:].rearrange("p a (b d) -> p (a b) d", d=D)
                        for tt in range(q0, q0 + w):
                            nc.tensor.matmul(pf[:, tt - q0, :], indt_sb[:, tt, :],
                                             xrb, start=True, stop=False)
                        for g2 in range(0, w, 2):
                            w2 = min(2, w - g2)
                            nc.tensor.matmul(
                                pf[:, g2:g2 + w2, :].rearrange("p a d -> p (a d)"),
                                i128_sb[:],
                                xsrc[:, q0 + g2:q0 + g2 + w2, :].rearrange(
                                    "p a d -> p (a d)"),
                                start=False, stop=True, skip_group_check=True)
                        u = upool.tile([128, QW, D], BF16, tag="u")
                        if os.environ.get("GAT_FB", "1") == "1":
                            # feat -> bf16 on ACT (frees PSUM fast; 2x DVE max)
                            fb = upool.tile([128, QW, D], BF16, tag="fb")
                            nc.scalar.activation(fb[:, 0:w, :], pf[:, 0:w, :],
                                                 mybir.ActivationFunctionType.Copy)
                            tmp = appool.tile([128, QW, D], BF16, tag="lrtmp")
                            nc.scalar.activation(tmp[:, 0:w, :], pf[:, 0:w, :],
                                                 mybir.ActivationFunctionType.Copy,
                                                 scale=SLOPE)
                            nc.vector.tensor_tensor(u[:, 0:w, :], fb[:, 0:w, :],
                                                    tmp[:, 0:w, :],
                                                    mybir.AluOpType.max)
                        else:
                            tmp = appool.tile([128, QW, D], BF16, tag="lrtmp")
                            nc.scalar.activation(tmp[:, 0:w, :], pf[:, 0:w, :],
                                                 mybir.ActivationFunctionType.Copy,
                                                 scale=SLOPE)
                            nc.vector.tensor_tensor(u[:, 0:w, :], pf[:, 0:w, :],
                                                    tmp[:, 0:w, :],
                                                    mybir.AluOpType.max)
                        ap_t = appool.tile([128, QW, D], BF16, tag="apt")
                        nc.vector.tensor_mul(ap_t[:, 0:w, :], u[:, 0:w, :],
                                             att_sb[:, l, 0:w, :])
                        e_sb = epool.tile([128, QW, 8], F32)
                        nc.vector.tensor_reduce(
                            e_sb[:, 0:w, :],
                            ap_t[:, 0:w, :].rearrange("p a (h c) -> p a h c", h=H),
                            axis=mybir.AxisListType.X, op=mybir.AluOpType.add)
                        nc.scalar.activation(ast[:, q0:q0 + w, 0:8], e_sb[:, 0:w, :],
                                             mybir.ActivationFunctionType.Exp)
                        nc.vector.tensor_mul(
                            ast[:, q0:q0 + w, 8:].rearrange("p a (h c) -> p a h c", h=H),
                            xsrc[:, q0:q0 + w, :].rearrange("p a (h c) -> p a h c", h=H),
                            ast[:, q0:q0 + w, 0:8].unsqueeze(-1).broadcast_to(
                                [128, w, H, C]))
                    for tt in range(t_blk):
                        nc.tensor.matmul(pb[:], ind_sb[:, tt, :], ast[:, tt, :],
                                         start=(tt == 0), stop=(tt == t_blk - 1))

                    dsb = smallp.tile([32, 8], F32, tag="dsb")
                    nc.scalar.activation(dsb[:], pb[:, 0:8],
                                         mybir.ActivationFunctionType.Identity,
                                         bias=eps_sb[:], scale=WSC)
                    dinv = smallp.tile([32, 8], F32, tag="dinv")
                    nc.vector.reciprocal(dinv[:], dsb[:])
                    outb = outbp.tile([32, D], F32)
                    nc.vector.tensor_mul(
                        outb[:].rearrange("p (h c) -> p h c", h=H),
                        pb[:, 8:].rearrange("p (h c) -> p h c", h=H),
                        dinv[:].unsqueeze(-1).broadcast_to([32, H, C]))
                    pt = ptrp.tile([128, 2, 32], F32)
                    nc.tensor.transpose(pt[:, 0, :], outb[:, 0:128], i32_sb[:])
                    nc.tensor.transpose(pt[:, 1, :], outb[:, 128:256], i32_sb[:])
                    for ch in range(2):
                        nc.vector.tensor_add(h_own[:, ch, 32 * b:32 * (b + 1)],
                                             h_own[:, ch, 32 * b:32 * (b + 1)],
                                             pt[:, ch, :])
                    if b % BPC == BPC - 1:
                        finalize_chunk(b // BPC)
            if nblk_run == 0:
                for k in range(NCHK):
                    finalize_chunk(k)
            if l == 0 and STAGE >= 5:
                nc.gpsimd.collective_compute(
                    "AllGather", mybir.AluOpType.bypass,
                    replica_groups=[list(range(NCORES))],
                    ins=[h_upd], outs=[h_all])

        # ---- final: y = h_own @ Wpred ----
        for k in range(3):
            w = 512 if k < 2 else NPC - 1024
            ps = pmm_tile()
            for ch in range(2):
                nc.tensor.matmul(ps[0:1, 0:w], bc_sb[:, 6 + ch:7 + ch],
                                 h_own[:, ch, 512 * k:512 * k + w],
                                 start=(ch == 0), stop=(ch == 1))
            ysb = drain.tile([1, 512], F32, tag="ysb")
            nc.scalar.activation(ysb[0:1, 0:w], ps[0:1, 0:w],
                                 mybir.ActivationFunctionType.Copy)
            nc.sync.dma_start(y_out[0:1, 512 * k:512 * k + w], ysb[0:1, 0:w])

    nc.compile()
    return nc


def _host_inputs(x, Wp, bp, Wl, Wr, att, bconv, Wpred, pm):
    xp = np.zeros((NP, F_IN), dtype=np.float32)
    xp[pm] = np.asarray(x, dtype=np.float32)
    xT = np.ascontiguousarray(xp.T)

    wl_p = np.zeros((128, L, 2, D), dtype=np.float32)
    wr_p = np.zeros((128, L, 2, D), dtype=np.float32)
    att_p = np.zeros((128, L, 4, D), dtype=np.float32)
    for l in range(L):
        for ch in range(2):
            wl_p[:, l, ch, :] = Wl[l][128 * ch:128 * (ch + 1), :]
            wr_p[:, l, ch, :] = Wr[l][128 * ch:128 * (ch + 1), :]
        a = att[l].reshape(H * C)
        for q in range(4):
            att_p[:, l, q, :] = a[None, :]

    bcols = np.zeros((128, 2 + 2 * L + 2), dtype=np.float32)
    for ch in range(2):
        bcols[:, ch] = bp[128 * ch:128 * (ch + 1)]
        for l in range(L):
            bcols[:, 2 + 2 * l + ch] = bconv[l][128 * ch:128 * (ch + 1)]
        bcols[:, 6 + ch] = Wpred[128 * ch:128 * (ch + 1), 0]

    shared = {
        "xT": xT,
        "wp": np.asarray(Wp, dtype=np.float32),
        "wl": (wl_p * WSC).astype(NF8), "wr": (wr_p * WSC).astype(NF8),
        "attrep": (att_p / WSC).astype(NBF),
        "bcols": bcols,
        "i128": np.eye(128, dtype=np.float32).astype(NBF),
        "i32": np.eye(32, dtype=np.float32),
        "epsc": np.full((32, 1), 1e-16, dtype=np.float32),
    }
    xTowns = [np.ascontiguousarray(xT[:, c * NPC:(c + 1) * NPC]) for c in range(NCORES)]
    return shared, xTowns


_CACHE = {}


def kernel(x, edge_index, batch, Wp, bp, Wl, Wr, att, bconv, Wpred, bpred,
           debug=False, _timing=None):
    x = np.asarray(x)
    edge_index = np.asarray(edge_index).astype(np.int64)
    batch = np.asarray(batch).astype(np.int64)

    t_blk, pm, cores = _prep_edges(edge_index)
    shared, xTowns = _host_inputs(np.asarray(x), np.asarray(Wp), np.asarray(bp),
                                  np.asarray(Wl), np.asarray(Wr), np.asarray(att),
                                  np.asarray(bconv), np.asarray(Wpred), pm)

    key = (t_blk, os.environ.get("GAT_STAGE", "99"),
           os.environ.get("GAT_LRELU", "1"), os.environ.get("GAT_GB", "2"),
           os.environ.get("GAT_FB", "1"))
    if key not in _CACHE:
        _CACHE[key] = _build(t_blk)
    nc = _CACHE[key]

    in_maps = []
    for c in range(NCORES):
        m = dict(shared)
        m["xTown"] = xTowns[c]
        m["srcw"] = cores[c]["srcw"]
        m["ind"] = cores[c]["ind_pm"]
        m["indt"] = cores[c]["indt_pm"]
        in_maps.append(m)

    kw = {k: v for k, v in (_timing or {}).items() if k != "result"}
    res = run_bass_kernel_spmd(nc, in_maps, list(range(NCORES)), **kw)
    if _timing is not None:
        _timing["result"] = res

    ycat = np.concatenate([res.results[c]["y_out"][0] for c in range(NCORES)])
    y_real = ycat[pm]
    sums = np.bincount(batch, weights=y_real.astype(np.float64), minlength=G)
    cnt = np.bincount(batch, minlength=G).astype(np.float64)
    out = sums / np.maximum(cnt, 1.0) + float(np.asarray(bpred).reshape(-1)[0])
    return out.astype(np.float32)[:, None]


# revision 22
# speedup vs baseline: 1.1078x; 1.1078x over previous
"""GATv2 (2-layer, 8-head) message-passing kernel for 8 Trainium2 NeuronCores.

v2: nodes are bin-packed into 40 dst-blocks of 32 nodes per core so every
block has <=1024 incoming edges -> uniform 8 tiles/block, ONE 1024-idx
dma_gather per block. Gather payload is plain xl (256 bf16 cols, 512B rows);
the LeakyReLU is computed directly on the Scalar engine (Lrelu, alpha=0.2),
so no linear-term augmentation is needed (the dst-side linear part need not
cancel: softmax is shift-invariant per dst, and we keep e exact anyway).
xr and the replicated h0 stay resident in SBUF; the xsrc add into PSUM is
done with 2 grouped identity matmuls (512 cols) instead of 8.

Self-contained: takes full (unsharded) inputs, returns the full output.
"""

import os
import numpy as np
from contextlib import ExitStack

import ml_dtypes
import concourse.bass as bass
import concourse.tile as tile
from concourse import bacc, mybir
from concourse.bass_utils import run_bass_kernel_spmd

N = 10000
E = 320000
F_IN = 7
D = 256
H = 8
C = 32
L = 2
G = 16
SLOPE = 0.2

NCORES = 8
NPC_REAL = 1250
NPC = 1280
NP = NCORES * NPC
BLK = 32
NBLK = NPC // BLK        # 40
PT = 128

F32 = mybir.dt.float32
BF16 = mybir.dt.bfloat16
I16 = mybir.dt.int16
FP8 = mybir.dt.float8e4
NBF = ml_dtypes.bfloat16
NF8 = ml_dtypes.float8_e4m3
WSC = 16.0   # fp8 weight scale (cancelled via att/WSC and denom scale)


def _prep_edges(edge_index: np.ndarray):
    """Bin-pack dst nodes into 40 blocks/core, relabel local node ids so
    block b = new-local nodes [32b, 32b+32). Returns (t_blk, pm, cores)."""
    src_g, dst_g = edge_index[0], edge_index[1]
    deg_all = np.zeros(N, dtype=np.int64)
    np.add.at(deg_all, dst_g, 1)

    pm = np.zeros(N, dtype=np.int64)      # original global -> padded new id
    for c in range(NCORES):
        deg = deg_all[c * NPC_REAL:(c + 1) * NPC_REAL]
        order = np.argsort(-deg, kind="stable")
        sums = np.zeros(NBLK)
        fill = np.zeros(NBLK, dtype=np.int64)
        newloc = np.zeros(NPC_REAL, dtype=np.int64)
        for n in order:
            cand = np.flatnonzero(fill < BLK)
            b = cand[np.argmin(sums[cand])]
            newloc[n] = b * BLK + fill[b]
            sums[b] += deg[n]
            fill[b] += 1
        pm[c * NPC_REAL:(c + 1) * NPC_REAL] = c * NPC + newloc

    owner = dst_g // NPC_REAL
    t_blk = 8
    per_core = []
    for c in range(NCORES):
        sel = owner == c
        es = pm[src_g[sel]]
        ed = pm[dst_g[sel]] - c * NPC
        cnt = np.bincount(ed // BLK, minlength=NBLK)
        t_blk = max(t_blk, int(np.ceil(cnt.max() / PT)))
        per_core.append((es, ed))

    spb = t_blk * PT                     # slots per block
    ne_slots = NBLK * spb
    cores = []
    for c in range(NCORES):
        es, ed = per_core[c]
        blk = ed // BLK
        order = np.argsort(blk, kind="stable")
        es, ed, blk = es[order], ed[order], blk[order]
        cnt = np.bincount(blk, minlength=NBLK)
        starts = np.zeros(NBLK, dtype=np.int64)
        starts[1:] = np.cumsum(cnt)[:-1]
        within = np.arange(len(es)) - starts[blk]
        slot = blk * spb + within

        src_slots = np.zeros(ne_slots, dtype=np.int64)
        dl32 = np.full(ne_slots, -1, dtype=np.int64)
        src_slots[slot] = es
        dl32[slot] = ed % BLK

        w16 = np.concatenate(
            [src_slots[b * spb:(b + 1) * spb].reshape(spb // 16, 16).T
             for b in range(NBLK)], axis=1).astype(np.int16)
        srcw = np.tile(w16, (8, 1))

        nt = NBLK * t_blk
        ind = np.zeros((ne_slots, BLK), dtype=np.float32)
        valid = dl32 >= 0
        ind[np.nonzero(valid)[0], dl32[valid]] = 1.0
        ind = ind.reshape(nt, PT, BLK)
        ind_pm = np.ascontiguousarray(ind.transpose(1, 0, 2)).astype(NBF)
        indt_pm = np.ascontiguousarray(
            ind.transpose(0, 2, 1).transpose(1, 0, 2)).astype(NBF)
        cores.append({"srcw": srcw, "ind_pm": ind_pm, "indt_pm": indt_pm})
    return t_blk, pm, cores


def _build(t_blk: int):
    STAGE = int(os.environ.get("GAT_STAGE", "99"))
    LRELU = os.environ.get("GAT_LRELU", "1") == "1"
    nt = NBLK * t_blk
    ne_slots = nt * PT
    QW = 4

    nc = bacc.Bacc("TRN2", target_bir_lowering=False, debug=False,
                   num_devices=NCORES)

    xT = nc.dram_tensor("xT", [F_IN, NP], BF16, kind="ExternalInput").ap()
    xTown = nc.dram_tensor("xTown", [F_IN, NPC], BF16, kind="ExternalInput").ap()
    wp = nc.dram_tensor("wp", [F_IN, D], BF16, kind="ExternalInput").ap()
    wl = nc.dram_tensor("wl", [128, L, 2, D], FP8, kind="ExternalInput").ap()
    wr = nc.dram_tensor("wr", [128, L, 2, D], FP8, kind="ExternalInput").ap()
    attrep = nc.dram_tensor("attrep", [128, L, QW, D], BF16, kind="ExternalInput").ap()
    bcols = nc.dram_tensor("bcols", [128, 2 + 2 * L + 2], F32, kind="ExternalInput").ap()
    i128 = nc.dram_tensor("i128", [128, 128], BF16, kind="ExternalInput").ap()
    i32 = nc.dram_tensor("i32", [32, 32], F32, kind="ExternalInput").ap()
    epsc = nc.dram_tensor("epsc", [32, 1], F32, kind="ExternalInput").ap()
    srcw_d = nc.dram_tensor("srcw", [128, ne_slots // 16], I16, kind="ExternalInput").ap()
    ind_d = nc.dram_tensor("ind", [128, nt, BLK], BF16, kind="ExternalInput").ap()
    indt_d = nc.dram_tensor("indt", [32, nt, PT], BF16, kind="ExternalInput").ap()

    xl_d = nc.dram_tensor("xl_d", [NP, D], BF16).ap()
    xr_d = nc.dram_tensor("xr_d", [NPC, D], BF16).ap()
    NCHK = 5
    CHN = NPC // NCHK
    h_upd = nc.dram_tensor("h_upd", [2, 128, NPC], FP8).ap()
    h_all = nc.dram_tensor("h_all", [NCORES, 2, 128, NPC], FP8,
                           addr_space="Shared").ap()

    y_out = nc.dram_tensor("y_out", [1, NPC], F32, kind="ExternalOutput").ap()

    with tile.TileContext(nc) as tc, ExitStack() as ctx:
        cpool = ctx.enter_context(tc.tile_pool(name="consts", bufs=1))
        stream = ctx.enter_context(tc.tile_pool(name="stream", bufs=4))
        drain = ctx.enter_context(tc.tile_pool(name="drain", bufs=4))
        gath = ctx.enter_context(tc.tile_pool(name="gath", bufs=4))
        indp = ctx.enter_context(tc.tile_pool(name="indp", bufs=3))
        indtp = ctx.enter_context(tc.tile_pool(name="indtp", bufs=3))
        xrp = ctx.enter_context(tc.tile_pool(name="xrp", bufs=3))
        upool = ctx.enter_context(tc.tile_pool(name="upool", bufs=3))
        appool = ctx.enter_context(tc.tile_pool(name="appool", bufs=3))
        epool = ctx.enter_context(tc.tile_pool(name="epool", bufs=3))
        astp = ctx.enter_context(tc.tile_pool(name="astp", bufs=4))
        smallp = ctx.enter_context(tc.tile_pool(name="smallp", bufs=3))
        outbp = ctx.enter_context(tc.tile_pool(name="outbp", bufs=3))

        pfeat = ctx.enter_context(tc.tile_pool(name="pfeat", bufs=2, space="PSUM"))
        pblk = ctx.enter_context(tc.tile_pool(name="pblk", bufs=3, space="PSUM"))
        ptrp = ctx.enter_context(tc.tile_pool(name="ptrp", bufs=1, space="PSUM"))

        def pmm_tile():
            t = pfeat.tile([128, 2, 512], F32, tag="pf")
            return t[:, 0, :]

        wp_sb = cpool.tile([F_IN, D], BF16)
        nc.sync.dma_start(wp_sb[:], wp)
        wl_sb = cpool.tile([128, L, 2, D], FP8)
        nc.sync.dma_start(wl_sb[:], wl)
        wr_sb = cpool.tile([128, L, 2, D], FP8)
        nc.sync.dma_start(wr_sb[:], wr)
        att_sb = cpool.tile([128, L, QW, D], BF16)
        nc.sync.dma_start(att_sb[:], attrep)
        bc_sb = cpool.tile([128, 2 + 2 * L + 2], F32)
        nc.sync.dma_start(bc_sb[:], bcols)
        i128_sb = cpool.tile([128, 128], BF16)
        nc.sync.dma_start(i128_sb[:], i128)
        i32_sb = cpool.tile([32, 32], F32)
        nc.sync.dma_start(i32_sb[:], i32)
        eps_sb = cpool.tile([32, 1], F32)
        nc.sync.dma_start(eps_sb[:], epsc)
        srcw_sb = cpool.tile([128, ne_slots // 16], I16)
        nc.sync.dma_start(srcw_sb[:], srcw_d)

        h_own = cpool.tile([128, 2, NPC], F32)    # own h, fp32, resident
        h_bf = cpool.tile([128, 2, NPC], FP8)    # own h, bf16 (xr stationary)
        h0_sb = cpool.tile([128, 2, NP], FP8)    # replicated h0 (layer-0 xl)

        def drain_ps(dst_ap, ps_ap, parity, bias=None):
            if bias is not None:
                if parity % 2 == 0:
                    nc.scalar.activation(dst_ap, ps_ap,
                                         mybir.ActivationFunctionType.Identity,
                                         bias=bias)
                else:
                    nc.vector.tensor_scalar_add(dst_ap, ps_ap, bias)
            elif parity % 2 == 0:
                nc.scalar.activation(dst_ap, ps_ap,
                                     mybir.ActivationFunctionType.Copy)
            else:
                nc.vector.tensor_copy(dst_ap, ps_ap)

        # ---- P0: h0 = x @ Wp + bp (replicated) -> h0_sb (bf16, SBUF) ----
        for b in range(NCORES):
            for k in range(4):
                xs = stream.tile([F_IN, 320], BF16, tag="xs")
                nc.sync.dma_start(xs[:], xT[:, b * NPC + 320 * k:b * NPC + 320 * (k + 1)])
                for ch in range(2):
                    ps = pmm_tile()
                    nc.tensor.matmul(ps[:, 0:320], wp_sb[:, 128 * ch:128 * (ch + 1)],
                                     xs[:], start=True, stop=True)
                    drain_ps(h0_sb[:, ch, b * NPC + 320 * k:b * NPC + 320 * (k + 1)],
                             ps[:, 0:320], b * 8 + k * 2 + ch,
                             bias=bc_sb[:, ch:ch + 1])
        # P0b: own slice -> h_own (f32) + h_bf (bf16)
        for k in range(4):
            xs = stream.tile([F_IN, 320], BF16, tag="xs")
            nc.sync.dma_start(xs[:], xTown[:, 320 * k:320 * (k + 1)])
            for ch in range(2):
                ps = pmm_tile()
                nc.tensor.matmul(ps[:, 0:320], wp_sb[:, 128 * ch:128 * (ch + 1)],
                                 xs[:], start=True, stop=True)
                drain_ps(h_own[:, ch, 320 * k:320 * (k + 1)], ps[:, 0:320], 0,
                         bias=bc_sb[:, ch:ch + 1])
                nc.vector.tensor_scalar_add(h_bf[:, ch, 320 * k:320 * (k + 1)],
                                            ps[:, 0:320], bc_sb[:, ch:ch + 1])

        # ---- layers ----
        for l in range(L if STAGE >= 2 else 0):
            # xl = h @ Wl  -> xl_d (node-major bf16)
            # jp <-> (bb, k): rows 256*jp = core bb = jp//5, chunk k = jp%5
            jp_order = (range(40) if l == 0 else
                        [5 * bb + k for k in range(5) for bb in range(8)])
            for jp in jp_order:
                ps2 = pfeat.tile([128, 2, 512], F32, tag="pf")
                if l == 0:
                    for o in range(2):
                        j = 2 * jp + o
                        nc.tensor.matmul(ps2[:, o, 0:D],
                                         h0_sb[:, :, 128 * j:128 * (j + 1)],
                                         wl_sb[:, l, :, :],
                                         start=True, stop=True,
                                         perf_mode=mybir.MatmulPerfMode.DoubleRow)
                else:
                    bb, lc = (2 * jp) // 10, ((2 * jp) % 10) * 128
                    ht = stream.tile([128, 2, 256], FP8, tag="ht")
                    nc.sync.dma_start(
                        ht[:], h_all[bb, :, :, lc:lc + 256].rearrange("c p n -> p c n"))
                    for o in range(2):
                        nc.tensor.matmul(ps2[:, o, 0:D],
                                         ht[:, :, 128 * o:128 * (o + 1)],
                                         wl_sb[:, l, :, :],
                                         start=True, stop=True,
                                         perf_mode=mybir.MatmulPerfMode.DoubleRow)
                xsb = drain.tile([128, 2, D], BF16, tag="xsb")
                drain_ps(xsb[:], ps2[:, :, 0:D], jp)
                row0 = 2 * jp * 128
                nc.sync.dma_start(
                    xl_d[row0:row0 + 256, :].rearrange("(a p) d -> p a d", p=128),
                    xsb[:])

            # xr = h_own @ Wr -> xr_d (node-major bf16, DRAM round-trip)
            for jo in range(NPC // 128 if STAGE >= 3 else 0):
                ps = pmm_tile()
                nc.tensor.matmul(ps[:, 0:D], h_bf[:, :, 128 * jo:128 * (jo + 1)],
                                 wr_sb[:, l, :, :],
                                 start=True, stop=True,
                                 perf_mode=mybir.MatmulPerfMode.DoubleRow)
                xrsb = drain.tile([128, D], BF16, tag="xrsb")
                drain_ps(xrsb[:], ps[:, 0:D], jo)
                nc.sync.dma_start(xr_d[128 * jo:128 * (jo + 1), :], xrsb[:])

            # edge sweep: gather 2 blocks per call (2048 idxs)
            nblk_run = NBLK if STAGE >= 4 else 0

            def finalize_chunk(k):
                """bias + bf16-cast + h_upd store for 256-node chunk k."""
                n0 = CHN * k
                for ch in range(2):
                    nc.vector.tensor_scalar_add(
                        h_own[:, ch, n0:n0 + CHN], h_own[:, ch, n0:n0 + CHN],
                        bc_sb[:, 2 + 2 * l + ch:3 + 2 * l + ch])
                if l == 0 and STAGE >= 5:
                    nc.scalar.activation(h_bf[:, 0, n0:n0 + CHN],
                                         h_own[:, 0, n0:n0 + CHN],
                                         mybir.ActivationFunctionType.Copy)
                    nc.vector.tensor_copy(h_bf[:, 1, n0:n0 + CHN],
                                          h_own[:, 1, n0:n0 + CHN])
                    nc.sync.dma_start(
                        h_upd[:, :, n0:n0 + CHN].rearrange("c p n -> p c n"),
                        h_bf[:, :, n0:n0 + CHN])

            BPC = NBLK // NCHK           # blocks per finalize chunk (8)
            GB = int(os.environ.get("GAT_GB", "2"))   # blocks per gather call
            for bp2 in range(nblk_run // GB):
                xsrc2 = gath.tile([128, GB * t_blk, D], BF16)
                for gs in range(0, GB * t_blk * PT, 2048):
                    gw = min(2048, GB * t_blk * PT - gs)
                    nc.gpsimd.dma_gather(
                        out_ap=xsrc2[:, gs // PT:(gs + gw) // PT, :],
                        in_ap=xl_d,
                        idxs_ap=srcw_sb[:, (bp2 * GB * t_blk * PT + gs) // 16:
                                        (bp2 * GB * t_blk * PT + gs + gw) // 16],
                        num_idxs=gw,
                        num_idxs_reg=gw,
                        elem_size=D,
                    )
                for sub in range(GB):
                    b = GB * bp2 + sub
                    xsrc = xsrc2[:, sub * t_blk:(sub + 1) * t_blk, :]
                    ind_sb = indp.tile([128, t_blk, BLK], BF16)
                    nc.sync.dma_start(ind_sb[:], ind_d[:, b * t_blk:(b + 1) * t_blk, :])
                    indt_sb = indtp.tile([32, t_blk, PT], BF16)
                    nc.sync.dma_start(indt_sb[:], indt_d[:, b * t_blk:(b + 1) * t_blk, :])
                    xrb_t = xrp.tile([32, D], BF16)
                    nc.sync.dma_start(xrb_t[:], xr_d[BLK * b:BLK * (b + 1), :])
                    xrb = xrb_t[:]

                    ast = astp.tile([128, t_blk, 8 + D], BF16)
                    pb = pblk.tile([32, 8 + D], F32)

                    for q0 in range(0, t_blk, QW):
                        w = min(QW, t_blk - q0)
                        pf0 = pfeat.tile([128, 2, 512], F32, tag="pf")
                        pf = pf0[:].rearrange("p a (b d) -> p (a b) d", d=D)
                        for tt in range(q0, q0 + w):
                            nc.tensor.matmul(pf[:, tt - q0, :], indt_sb[:, tt, :],
                                             xrb, start=True, stop=False)
                        for g2 in range(0, w, 2):
                            w2 = min(2, w - g2)
                            nc.tensor.matmul(
                                pf[:, g2:g2 + w2, :].rearrange("p a d -> p (a d)"),
                                i128_sb[:],
                                xsrc[:, q0 + g2:q0 + g2 + w2, :].rearrange(
                                    "p a d -> p (a d)"),
                                start=False, stop=True, skip_group_check=True)
                        u = upool.tile([128, QW, D], BF16, tag="u")
                        if os.environ.get("GAT_FB", "1") == "1":
                            # feat -> bf16 on ACT (frees PSUM fast; 2x DVE max)
                            fb = upool.tile([128, QW, D], BF16, tag="fb")
                            nc.scalar.activation(fb[:, 0:w, :], pf[:, 0:w, :],
                                                 mybir.ActivationFunctionType.Copy)
                            tmp = appool.tile([128, QW, D], BF16, tag="lrtmp")
                            nc.scalar.activation(tmp[:, 0:w, :], pf[:, 0:w, :],
                                                 mybir.ActivationFunctionType.Copy,
                                                 scale=SLOPE)
                            nc.vector.tensor_tensor(u[:, 0:w, :], fb[:, 0:w, :],
                                                    tmp[:, 0:w, :],
                                                    mybir.AluOpType.max)
                        else:
                            tmp = appool.tile([128, QW, D], BF16, tag="lrtmp")
                            nc.scalar.activation(tmp[:, 0:w, :], pf[:, 0:w, :],
                                                 mybir.ActivationFunctionType.Copy,
                                                 scale=SLOPE)
                            nc.vector.tensor_tensor(u[:, 0:w, :], pf[:, 0:w, :],
                                                    tmp[:, 0:w, :],
                                                    mybir.AluOpType.max)
                        ap_t = appool.tile([128, QW, D], BF16, tag="apt")
                        nc.vector.tensor_mul(ap_t[:, 0:w, :], u[:, 0:w, :],
                                             att_sb[:, l, 0:w, :])
                        e_sb = epool.tile([128, QW, 8], F32)
                        nc.vector.tensor_reduce(
                            e_sb[:, 0:w, :],
                            ap_t[:, 0:w, :].rearrange("p a (h c) -> p a h c", h=H),
                            axis=mybir.AxisListType.X, op=mybir.AluOpType.add)
                        nc.scalar.activation(ast[:, q0:q0 + w, 0:8], e_sb[:, 0:w, :],
                                             mybir.ActivationFunctionType.Exp)
                        nc.vector.tensor_mul(
                            ast[:, q0:q0 + w, 8:].rearrange("p a (h c) -> p a h c", h=H),
                            xsrc[:, q0:q0 + w, :].rearrange("p a (h c) -> p a h c", h=H),
                            ast[:, q0:q0 + w, 0:8].unsqueeze(-1).broadcast_to(
                                [128, w, H, C]))
                    for tt in range(t_blk):
                        nc.tensor.matmul(pb[:], ind_sb[:, tt, :], ast[:, tt, :],
                                         start=(tt == 0), stop=(tt == t_blk - 1))

                    dsb = smallp.tile([32, 8], F32, tag="dsb")
                    nc.scalar.activation(dsb[:], pb[:, 0:8],
                                         mybir.ActivationFunctionType.Identity,
                                         bias=eps_sb[:], scale=WSC)
                    dinv = smallp.tile([32, 8], F32, tag="dinv")
                    nc.vector.reciprocal(dinv[:], dsb[:])
                    outb = outbp.tile([32, D], F32)
                    nc.vector.tensor_mul(
                        outb[:].rearrange("p (h c) -> p h c", h=H),
                        pb[:, 8:].rearrange("p (h c) -> p h c", h=H),
                        dinv[:].unsqueeze(-1).broadcast_to([32, H, C]))
                    pt = ptrp.tile([128, 2, 32], F32)
                    nc.tensor.transpose(pt[:, 0, :], outb[:, 0:128], i32_sb[:])
                    nc.tensor.transpose(pt[:, 1, :], outb[:, 128:256], i32_sb[:])
                    for ch in range(2):
                        nc.vector.tensor_add(h_own[:, ch, 32 * b:32 * (b + 1)],
                                             h_own[:, ch, 32 * b:32 * (b + 1)],
                                             pt[:, ch, :])
                    if b % BPC == BPC - 1:
                        finalize_chunk(b // BPC)
            if nblk_run == 0:
                for k in range(NCHK):
                    finalize_chunk(k)
            if l == 0 and STAGE >= 5:
                nc.gpsimd.collective_compute(
                    "AllGather", mybir.AluOpType.bypass,
                    replica_groups=[list(range(NCORES))],
                    ins=[h_upd], outs=[h_all])

        # ---- final: y = h_own @ Wpred ----
        for k in range(3):
            w = 512 if k < 2 else NPC - 1024
            ps = pmm_tile()
            for ch in range(2):
                nc.tensor.matmul(ps[0:1, 0:w], bc_sb[:, 6 + ch:7 + ch],
                                 h_own[:, ch, 512 * k:512 * k + w],
                                 start=(ch == 0), stop=(ch == 1))
            ysb = drain.tile([1, 512], F32, tag="ysb")
            nc.scalar.activation(ysb[0:1, 0:w], ps[0:1, 0:w],
                                 mybir.ActivationFunctionType.Copy)
            nc.sync.dma_start(y_out[0:1, 512 * k:512 * k + w], ysb[0:1, 0:w])

    nc.compile()
    return nc


def _host_inputs(x, Wp, bp, Wl, Wr, att, bconv, Wpred, pm):
    xp = np.zeros((NP, F_IN), dtype=np.float32)
    xp[pm] = np.asarray(x, dtype=np.float32)
    xT = np.ascontiguousarray(xp.T).astype(NBF)

    wl_p = np.zeros((128, L, 2, D), dtype=np.float32)
    wr_p = np.zeros((128, L, 2, D), dtype=np.float32)
    att_p = np.zeros((128, L, 4, D), dtype=np.float32)
    for l in range(L):
        for ch in range(2):
            wl_p[:, l, ch, :] = Wl[l][128 * ch:128 * (ch + 1), :]
            wr_p[:, l, ch, :] = Wr[l][128 * ch:128 * (ch + 1), :]
        a = att[l].reshape(H * C)
        for q in range(4):
            att_p[:, l, q, :] = a[None, :]

    bcols = np.zeros((128, 2 + 2 * L + 2), dtype=np.float32)
    for ch in range(2):
        bcols[:, ch] = bp[128 * ch:128 * (ch + 1)]
        for l in range(L):
            bcols[:, 2 + 2 * l + ch] = bconv[l][128 * ch:128 * (ch + 1)]
        bcols[:, 6 + ch] = Wpred[128 * ch:128 * (ch + 1), 0]

    shared = {
        "xT": xT,
        "wp": np.asarray(Wp, dtype=np.float32).astype(NBF),
        "wl": (wl_p * WSC).astype(NF8), "wr": (wr_p * WSC).astype(NF8),
        "attrep": (att_p / WSC).astype(NBF),
        "bcols": bcols,
        "i128": np.eye(128, dtype=np.float32).astype(NBF),
        "i32": np.eye(32, dtype=np.float32),
        "epsc": np.full((32, 1), 1e-16, dtype=np.float32),
    }
    xTowns = [np.ascontiguousarray(xT[:, c * NPC:(c + 1) * NPC]) for c in range(NCORES)]
    return shared, xTowns


_CACHE = {}


def kernel(x, edge_index, batch, Wp, bp, Wl, Wr, att, bconv, Wpred, bpred,
           debug=False, _timing=None):
    x = np.asarray(x)
    edge_index = np.asarray(edge_index).astype(np.int64)
    batch = np.asarray(batch).astype(np.int64)

    t_blk, pm, cores = _prep_edges(edge_index)
    shared, xTowns = _host_inputs(np.asarray(x), np.asarray(Wp), np.asarray(bp),
                                  np.asarray(Wl), np.asarray(Wr), np.asarray(att),
                                  np.asarray(bconv), np.asarray(Wpred), pm)

    key = (t_blk, os.environ.get("GAT_STAGE", "99"),
           os.environ.get("GAT_LRELU", "1"), os.environ.get("GAT_GB", "2"),
           os.environ.get("GAT_FB", "1"))
    if key not in _CACHE:
        _CACHE[key] = _build(t_blk)
    nc = _CACHE[key]

    in_maps = []
    for c in range(NCORES):
        m = dict(shared)
        m["xTown"] = xTowns[c]
        m["srcw"] = cores[c]["srcw"]
        m["ind"] = cores[c]["ind_pm"]
        m["indt"] = cores[c]["indt_pm"]
        in_maps.append(m)

    kw = {k: v for k, v in (_timing or {}).items() if k != "result"}
    res = run_bass_kernel_spmd(nc, in_maps, list(range(NCORES)), **kw)
    if _timing is not None:
        _timing["result"] = res

    ycat = np.concatenate([res.results[c]["y_out"][0] for c in range(NCORES)])
    y_real = ycat[pm]
    sums = np.bincount(batch, weights=y_real.astype(np.float64), minlength=G)
    cnt = np.bincount(batch, minlength=G).astype(np.float64)
    out = sums / np.maximum(cnt, 1.0) + float(np.asarray(bpred).reshape(-1)[0])
    return out.astype(np.float32)[:, None]
